# revision 18
# baseline (speedup 1.0000x reference)
"""GCN (2-layer, sym-norm + self-loops, BN, segment-max pool) on 8 TRN2 cores.

Strategy (v2):
  - nodes sharded 8x6250 (contiguous); edges sharded by dst owner.
  - layer-1 aggregation reassociated: AX = A_norm @ X (128-dim), then
    H1 = AX @ W1 (per-node transform on local shard).
  - gathers of table rows (X / G) via gpsimd.dma_gather (int16 idx, table
    split in two 25000-row halves); aggregation via one-hot mask matmuls
    into PSUM per 128-dst block; self-loops via diag matmul (no gather).
  - v2: gathers round-robin over 4 SWDGE queues (Q7 core-pair per queue
    pipelines ~2 deep -> ~2x); all mask/matmul work in bf16 (fp32 psum);
    BN1 folded into W2 (G = relu(H1) @ (a1*W2) + sh1@W2), G/AllGather in
    bf16.
  - BN stats + final per-graph max via collectives (AllReduce), G via
    AllGather.
  - program is identical on all 8 cores; all per-core variation is data.
"""
import sys

if "/opt/trn_rl_repo" not in sys.path:
    sys.path.insert(0, "/opt/trn_rl_repo")

import numpy as np
import ml_dtypes

import concourse.bacc as bacc
import concourse.bass as bass
import concourse.tile as tile
from concourse import bass_utils, mybir
from concourse.masks import make_identity

N = 50000
E = 800000
NG = 512
DIN, DHID, DEMB = 128, 256, 128
EPS = 1e-5
NC = 8
NLOC = N // NC          # 6250
NBLK = (NLOC + 127) // 128   # 49
LASTV = NLOC - (NBLK - 1) * 128  # 106 valid rows in last block
HALF = 25000
P = 128
NQ = 4                  # SWDGE queues for gather ping-pong

_cache = {}


def _wrap16(v):
    """[k*128] int16 -> [128, 8*k] in the 16-partition-wrapped, x8-replicated
    layout dma_gather wants (element j -> partition j%16, col j//16)."""
    v = np.asarray(v, dtype=np.int16)
    k = v.size // 16
    w = v.reshape(k, 16).T          # [16, k]
    return np.tile(w, (8, 1))       # [128, k]


def _prep(x, edge_index, batch):
    """Host-side graph preprocessing -> per-core data + program constants."""
    src = np.asarray(edge_index[0], dtype=np.int64)
    dst = np.asarray(edge_index[1], dtype=np.int64)
    batch = np.asarray(batch, dtype=np.int64)

    deg = np.bincount(dst, minlength=N).astype(np.float64) + 1.0
    dinv = 1.0 / np.sqrt(deg)
    norm = (dinv[src] * dinv[dst]).astype(np.float32)

    core = dst // NLOC
    dloc = dst % NLOC
    blk = dloc // P
    dstl = (dloc % P).astype(np.float32)
    half = (src >= HALF).astype(np.int64)
    srel = (src - HALF * half).astype(np.int64)

    # group id: (core, blk, half); sort edges into groups
    gid = (core * NBLK + blk) * 2 + half
    order = np.argsort(gid, kind="stable")
    gid_s = gid[order]
    counts = np.bincount(gid, minlength=NC * NBLK * 2).reshape(NC, NBLK, 2)
    # chunks per (blk, half): max over cores, in units of 128 edges
    cc = (counts.max(axis=0) + P - 1) // P        # [NBLK, 2]
    totch = int(cc.sum())                          # chunks per core (uniform)
    TOT = totch * P

    # per-group slot offsets within the padded per-core layout
    goff = np.zeros((NBLK, 2), dtype=np.int64)
    run = 0
    for b in range(NBLK):
        for h in range(2):
            goff[b, h] = run
            run += int(cc[b, h]) * P

    srel_s = srel[order]
    dstl_s = dstl[order]
    norm_s = norm[order]
    core_s = core[order]
    blk_s = blk[order]
    half_s = half[order]
    # rank within group
    grp_start = np.zeros(NC * NBLK * 2, dtype=np.int64)
    np.cumsum(counts.reshape(-1)[:-1], out=grp_start[1:])
    rank = np.arange(E, dtype=np.int64) - grp_start[gid_s]

    slot = goff[blk_s, half_s] + rank              # slot within its core
    src16 = np.zeros((NC, TOT), dtype=np.int16)
    dstlA = np.zeros((NC, TOT), dtype=np.float32)
    normA = np.zeros((NC, TOT), dtype=np.float32)
    src16[core_s, slot] = srel_s.astype(np.int16)
    dstlA[core_s, slot] = dstl_s
    normA[core_s, slot] = norm_s

    # device layouts
    dstl_host = dstlA.reshape(NC, totch, P).transpose(0, 2, 1).copy()   # [NC,128,totch]
    norm_host = normA.reshape(NC, totch, P).transpose(0, 2, 1).copy()
    src_host = np.zeros((NC, P, 8 * totch), dtype=np.int16)
    ch0 = np.concatenate([[0], np.cumsum(cc.reshape(-1))]).astype(np.int64)
    for g in range(NBLK * 2):
        c0, c1 = int(ch0[g]), int(ch0[g + 1])
        if c0 == c1:
            continue
        for c in range(NC):
            seg = src16[c, c0 * P:c1 * P]
            src_host[c, :, c0 * 8:c1 * 8] = _wrap16(seg)

    # self-loop diag values 1/deg per local block
    dinv2 = np.zeros((NC, P, NBLK), dtype=np.float32)
    for c in range(NC):
        v = (1.0 / deg[c * NLOC:(c + 1) * NLOC]).astype(np.float32)
        pad = np.zeros(NBLK * P, dtype=np.float32)
        pad[:NLOC] = v
        dinv2[c] = pad.reshape(NBLK, P).T

    # ---- segment-max slots ----
    gcounts = np.bincount(batch, minlength=NG)
    gstart = np.concatenate([[0], np.cumsum(gcounts)]).astype(np.int64)
    slots = [[] for _ in range(NC)]               # (graph, lo_local, hi_local)
    for g in range(NG):
        a, b2 = int(gstart[g]), int(gstart[g + 1])
        if a == b2:
            continue
        c0, c1 = a // NLOC, (b2 - 1) // NLOC
        for c in range(c0, c1 + 1):
            lo = max(a, c * NLOC) - c * NLOC
            hi = min(b2, (c + 1) * NLOC) - c * NLOC
            assert hi - lo <= 2 * P, "graph piece exceeds 256 nodes"
            slots[c].append((g, lo, hi))
    SMAX = max(len(s) for s in slots)
    scount = np.ones(SMAX, dtype=np.int64)
    for c in range(NC):
        for s, (g, lo, hi) in enumerate(slots[c]):
            scount[s] = max(scount[s], (hi - lo + P - 1) // P)
    sch0 = np.concatenate([[0], np.cumsum(scount)]).astype(np.int64)
    TC = int(scount.sum())
    selidx = np.zeros((NC, P, 8 * TC), dtype=np.int16)
    offe = np.zeros((NC, P, 1), dtype=np.int32)
    for c in range(NC):
        for s in range(P):
            offe[c, s, 0] = 512 + s
        for s in range(SMAX):
            g, lo, hi = slots[c][s] if s < len(slots[c]) else (None, 0, 1)
            if g is not None:
                offe[c, s, 0] = g
            for j in range(int(scount[s])):
                a = lo + j * P
                idx = np.full(P, lo, dtype=np.int16)
                n = max(0, min(hi - a, P))
                if n > 0:
                    idx[:n] = np.arange(a, a + n, dtype=np.int16)
                col = int(sch0[s]) + j
                selidx[c, :, 8 * col:8 * (col + 1)] = _wrap16(idx)

    # ---- second edge grouping for layer 2 (tables split at HLOC within
    # each owner shard so the two AllGather halves can be separate) ----
    HLOC = NLOC // 2
    srcc = src // NLOC
    srcr = src % NLOC
    half2 = (srcr >= HLOC).astype(np.int64)
    srel2 = (srcc * HLOC + srcr - HLOC * half2).astype(np.int64)
    gid2 = (core * NBLK + blk) * 2 + half2
    order2 = np.argsort(gid2, kind="stable")
    gid2_s = gid2[order2]
    counts2 = np.bincount(gid2, minlength=NC * NBLK * 2).reshape(NC, NBLK, 2)
    cc2 = (counts2.max(axis=0) + P - 1) // P
    totch2 = int(cc2.sum())
    TOT2 = totch2 * P
    goff2 = np.zeros((NBLK, 2), dtype=np.int64)
    run2 = 0
    for b in range(NBLK):
        for h in range(2):
            goff2[b, h] = run2
            run2 += int(cc2[b, h]) * P
    grp2_start = np.zeros(NC * NBLK * 2, dtype=np.int64)
    np.cumsum(counts2.reshape(-1)[:-1], out=grp2_start[1:])
    rank2 = np.arange(E, dtype=np.int64) - grp2_start[gid2_s]
    slot2 = goff2[blk_s2 := blk[order2], half2[order2]] + rank2
    core_s2 = core[order2]
    src16_2 = np.zeros((NC, TOT2), dtype=np.int16)
    dstl2A = np.zeros((NC, TOT2), dtype=np.float32)
    norm2A = np.zeros((NC, TOT2), dtype=np.float32)
    src16_2[core_s2, slot2] = srel2[order2].astype(np.int16)
    dstl2A[core_s2, slot2] = dstl[order2]
    norm2A[core_s2, slot2] = norm[order2]
    dstl2_host = dstl2A.reshape(NC, totch2, P).transpose(0, 2, 1).copy()
    norm2_host = norm2A.reshape(NC, totch2, P).transpose(0, 2, 1).copy()
    src2_host = np.zeros((NC, P, 8 * totch2), dtype=np.int16)
    ch02 = np.concatenate([[0], np.cumsum(cc2.reshape(-1))]).astype(np.int64)
    for g in range(NBLK * 2):
        c0, c1 = int(ch02[g]), int(ch02[g + 1])
        if c0 == c1:
            continue
        for c in range(NC):
            seg = src16_2[c, c0 * P:c1 * P]
            src2_host[c, :, c0 * 8:c1 * 8] = _wrap16(seg)

    meta = {
        "cc": cc, "totch": totch, "SMAX": SMAX, "scount": scount,
        "src_host": src_host, "dstl_host": dstl_host, "norm_host": norm_host,
        "cc2": cc2, "totch2": totch2, "src2_host": src2_host,
        "dstl2_host": dstl2_host, "norm2_host": norm2_host,
        "dinv2": dinv2, "selidx": selidx, "offe": offe,
        "gcounts": gcounts,
    }
    return meta


def _build(cc, totch, cc2, totch2, SMAX, scount):
    """Build the SPMD program. cc: [NBLK,2] chunk counts, identical across
    cores."""
    TC = int(scount.sum())
    sch0 = np.concatenate([[0], np.cumsum(scount)]).astype(np.int64)
    nc = bacc.Bacc("TRN2", target_bir_lowering=False, debug=False,
                   enable_asserts=True, num_devices=NC, num_swdge_queues=NQ)
    f32, bf16, i16, i32 = (mybir.dt.float32, mybir.dt.bfloat16,
                           mybir.dt.int16, mybir.dt.int32)

    def din(name, shape, dt=f32):
        return nc.dram_tensor(name, shape, dt, kind="ExternalInput").ap()

    x_tab = din("x_tab", [N, DIN])
    x_own = din("x_own", [NLOC, DIN], bf16)
    w1 = din("w1", [DIN, DHID])
    w2p = din("w2p", [P, DHID])            # w2p[p, j*128+c] = W2[j*128+p, c]
    b1h = din("b1h", [P, 2])
    b2b = din("b2b", [P, DEMB])
    g1h = din("g1h", [P, 2])
    bt1h = din("bt1h", [P, 2])
    g2r = din("g2r", [1, DEMB])
    bt2r = din("bt2r", [1, DEMB])
    iota_b = din("iota_b", [P, P], bf16)
    onesel = din("onesel", [P, 2])
    onesrow = din("onesrow", [1, P])
    onesb = din("onesb", [1, P], bf16)
    src16 = din("src16", [P, 8 * totch], i16)
    dstlB = din("dstlB", [P, totch], bf16)
    normA = din("normA", [P, totch])
    src16b = din("src16b", [P, 8 * totch2], i16)
    dstlB2 = din("dstlB2", [P, totch2], bf16)
    normB2 = din("normB2", [P, totch2], bf16)
    dinv2b = din("dinv2b", [P, NBLK], bf16)
    selidx = din("selidx", [P, 8 * TC], i16)
    offe = din("offe", [P, 1], i32)

    out = nc.dram_tensor("out", [NG, DEMB], f32, kind="ExternalOutput").ap()

    # chunk column offsets per (blk, half)
    ch0 = np.concatenate([[0], np.cumsum(cc.reshape(-1))]).astype(np.int64)
    ch02 = np.concatenate([[0], np.cumsum(cc2.reshape(-1))]).astype(np.int64)
    HLOC = NLOC // 2

    qctr = [0]

    def next_q():
        q = qctr[0] % NQ
        qctr[0] += 1
        return q

    with tile.TileContext(nc) as tc:
        with (
            tc.tile_pool(name="const", bufs=1) as cp,
            tc.tile_pool(name="meta", bufs=1) as mp,
            tc.tile_pool(name="store", bufs=1) as stp,
            tc.tile_pool(name="gath", bufs=3) as gpool,
            tc.tile_pool(name="gathb", bufs=2) as gbp,
            tc.tile_pool(name="mask", bufs=2) as mkp,
            tc.tile_pool(name="work", bufs=3) as wp,
            tc.tile_pool(name="ps_agg", bufs=2, space="PSUM") as ps_agg,
            tc.tile_pool(name="ps_mm", bufs=2, space="PSUM") as ps_mm,
            tc.tile_pool(name="ps_tr", bufs=2, space="PSUM") as ps_tr,
            tc.tile_pool(name="dram", bufs=1, space="DRAM") as dr,
        ):
            # ---------- constants / meta ----------
            src_sb = mp.tile([P, 8 * totch], i16)
            nc.sync.dma_start(out=src_sb[:], in_=src16[:])
            dstl_sb = mp.tile([P, totch], bf16)
            nc.sync.dma_start(out=dstl_sb[:], in_=dstlB[:])
            norm_sb = mp.tile([P, totch], f32)
            nc.sync.dma_start(out=norm_sb[:], in_=normA[:])
            src2_sb = mp.tile([P, 8 * totch2], i16)
            nc.sync.dma_start(out=src2_sb[:], in_=src16b[:])
            dstl2_sb = mp.tile([P, totch2], bf16)
            nc.sync.dma_start(out=dstl2_sb[:], in_=dstlB2[:])
            norm2b_sb = mp.tile([P, totch2], bf16)
            nc.sync.dma_start(out=norm2b_sb[:], in_=normB2[:])
            dinv2_sb = mp.tile([P, NBLK], bf16)
            nc.sync.dma_start(out=dinv2_sb[:], in_=dinv2b[:])
            sel_sb = mp.tile([P, 8 * TC], i16)
            nc.sync.dma_start(out=sel_sb[:], in_=selidx[:])
            offe_sb = mp.tile([P, 1], i32)
            nc.sync.dma_start(out=offe_sb[:], in_=offe[:])

            w1f = cp.tile([P, DHID], f32)
            nc.sync.dma_start(out=w1f[:], in_=w1[:])
            w1b = cp.tile([P, DHID], bf16)
            nc.vector.tensor_copy(out=w1b[:], in_=w1f[:])
            w2_sb = cp.tile([P, DHID], f32)
            nc.sync.dma_start(out=w2_sb[:], in_=w2p[:])
            b1_sb = cp.tile([P, 2], f32)
            nc.sync.dma_start(out=b1_sb[:], in_=b1h[:])
            b2b_sb = cp.tile([P, DEMB], f32)
            nc.sync.dma_start(out=b2b_sb[:], in_=b2b[:])
            g1_sb = cp.tile([P, 2], f32)
            nc.sync.dma_start(out=g1_sb[:], in_=g1h[:])
            bt1_sb = cp.tile([P, 2], f32)
            nc.sync.dma_start(out=bt1_sb[:], in_=bt1h[:])
            g2_sb = cp.tile([1, DEMB], f32)
            nc.sync.dma_start(out=g2_sb[:], in_=g2r[:])
            bt2_sb = cp.tile([1, DEMB], f32)
            nc.sync.dma_start(out=bt2_sb[:], in_=bt2r[:])
            iotab_sb = cp.tile([P, P], bf16)
            nc.sync.dma_start(out=iotab_sb[:], in_=iota_b[:])
            ones_sb = cp.tile([P, 2], f32)
            nc.sync.dma_start(out=ones_sb[:], in_=onesel[:])
            onesr_sb = cp.tile([1, P], f32)
            nc.sync.dma_start(out=onesr_sb[:], in_=onesrow[:])
            onesb_sb = cp.tile([1, P], bf16)
            nc.sync.dma_start(out=onesb_sb[:], in_=onesb[:])
            ident = cp.tile([P, P], f32)
            make_identity(nc, ident[:])
            ident_b = cp.tile([P, P], bf16)
            nc.vector.tensor_copy(out=ident_b[:], in_=ident[:])

            # ---------- persistent stores ----------
            s1a = stp.tile([P, 2 * NBLK], f32, tag="s1a")
            s2a = stp.tile([P, 2 * NBLK], f32, tag="s2a")
            r_st0 = stp.tile([P, NBLK * P], bf16, tag="r0")
            r_st1 = stp.tile([P, NBLK * P], bf16, tag="r1")
            r_st = [r_st0, r_st1]
            h2_st = stp.tile([P, NBLK * P], f32)
            stats_sb = stp.tile([P, 4], f32)
            bn2acc = stp.tile([1, 2 * DEMB], f32)
            segcols = stp.tile([P, P], f32)
            w2s = stp.tile([P, DHID], bf16)
            shw = stp.tile([1, P], bf16)

            # ---------- DRAM internals ----------
            gshardA = dr.tile([NLOC // 2, DEMB], bf16)
            gshardB = dr.tile([NLOC // 2, DEMB], bf16)
            g_tabA = dr.tile([N // 2, DEMB], bf16)
            g_tabB = dr.tile([N // 2, DEMB], bf16)
            st1_in = dr.tile([P, 4], f32)
            st1_out = dr.tile([P, 4], f32)
            st2_in = dr.tile([1, 2 * DEMB], f32)
            st2_out = dr.tile([1, 2 * DEMB], f32)
            segtab = dr.tile([640, DEMB], f32)
            arout = dr.tile([NG, DEMB], f32)
            h2loc = dr.tile([NLOC, DEMB], f32)

            def agg_layer(layer):
                """Aggregation for one layer. layer 1: psum AXT [f,d]; layer
                2: psum AG [d,f] node-major. Both use bf16 operands."""
                lcc = cc if layer == 1 else cc2
                lch0 = ch0 if layer == 1 else ch02
                lsrc = src_sb if layer == 1 else src2_sb
                ldstl = dstl_sb if layer == 1 else dstl2_sb
                for b in range(NBLK):
                    nch = int(lcc[b, 0] + lcc[b, 1])
                    # gather this block's edges (both halves) into one tile
                    if layer == 1:
                        gt = gpool.tile([P, nch * P], f32, tag="gt",
                                        name="gt", bufs=4)
                    else:
                        gt = gpool.tile([P, nch * P], bf16, tag="gt2",
                                        name="gt2", bufs=4)
                    for h in range(2):
                        ng = int(lcc[b, h])
                        if ng == 0:
                            continue
                        c0 = int(lch0[2 * b + h])
                        crel = c0 - int(lch0[2 * b])
                        if layer == 1:
                            base = 0 if h == 0 else HALF
                            tab = x_tab[base:base + HALF, :]
                        else:
                            tab = (g_tabA if h == 0 else g_tabB)[:]
                        nc.gpsimd.dma_gather(
                            out_ap=gt[:, crel * P:(crel + ng) * P].rearrange(
                                "p (c e) -> p c e", e=P),
                            in_ap=tab,
                            idxs_ap=lsrc[:, c0 * 8:(c0 + ng) * 8],
                            num_idxs=ng * P,
                            num_idxs_reg=ng * P,
                            elem_size=P,
                            single_packet=False,
                            queue_num=next_q(),
                        )
                    # batched bf16 mask build + per-edge norm fold
                    c0b = int(lch0[2 * b])
                    mk = mkp.tile([P, nch * P], bf16, tag="mk")
                    iap = iotab_sb[:]
                    iota_rep = bass.AP(iap.tensor, iap.offset,
                                       [iap.ap[0], [0, nch], [1, P]])
                    nc.vector.tensor_tensor(
                        out=mk[:].rearrange("p (c e) -> p c e", e=P),
                        in0=iota_rep,
                        in1=ldstl[:, c0b:c0b + nch].to_broadcast([P, nch, P]),
                        op=mybir.AluOpType.is_equal)
                    if layer == 1:
                        gtb = gbp.tile([P, nch * P], bf16, tag="gtb")
                        nc.vector.tensor_tensor(
                            out=gtb[:].rearrange("p (c e) -> p c e", e=P),
                            in0=gt[:].rearrange("p (c e) -> p c e", e=P),
                            in1=norm_sb[:, c0b:c0b + nch].to_broadcast(
                                [P, nch, P]),
                            op=mybir.AluOpType.mult)
                    else:
                        gtb = gt
                        nc.vector.tensor_tensor(
                            out=gtb[:].rearrange("p (c e) -> p c e", e=P),
                            in0=gtb[:].rearrange("p (c e) -> p c e", e=P),
                            in1=norm2b_sb[:, c0b:c0b + nch].to_broadcast(
                                [P, nch, P]),
                            op=mybir.AluOpType.mult)
                    aps = ps_agg.tile([P, P], f32, tag="agg", space="PSUM")
                    for j in range(nch):
                        if layer == 1:
                            nc.tensor.matmul(
                                out=aps[:], lhsT=gtb[:, j * P:(j + 1) * P],
                                rhs=mk[:, j * P:(j + 1) * P],
                                start=(j == 0), stop=False)
                        else:
                            nc.tensor.matmul(
                                out=aps[:], lhsT=mk[:, j * P:(j + 1) * P],
                                rhs=gtb[:, j * P:(j + 1) * P],
                                start=(j == 0), stop=False)
                    # self-loop: diag(1/deg) @ own rows (bf16)
                    dg = mkp.tile([P, P], bf16, tag="dg")
                    nc.vector.tensor_tensor(
                        out=dg[:], in0=ident_b[:],
                        in1=dinv2_sb[:, b:b + 1].to_broadcast([P, P]),
                        op=mybir.AluOpType.mult)
                    nv = P if b < NBLK - 1 else LASTV
                    if layer == 1:
                        xo_b = wp.tile([P, P], bf16, tag="xob")
                        nc.sync.dma_start(out=xo_b[:nv, :],
                                          in_=x_own[b * P:b * P + nv, :])
                        nc.tensor.matmul(out=aps[:], lhsT=xo_b[:nv, :],
                                         rhs=dg[:nv, :], start=False,
                                         stop=True)
                    else:
                        xo_b = wp.tile([P, P], bf16, tag="xob")
                        lo, hi = b * P, b * P + nv
                        if lo < HLOC:
                            n1 = min(hi, HLOC) - lo
                            nc.sync.dma_start(out=xo_b[:n1, :],
                                              in_=gshardA[lo:lo + n1, :])
                        if hi > HLOC:
                            o = max(lo, HLOC)
                            n2 = hi - o
                            nc.sync.dma_start(
                                out=xo_b[o - lo:o - lo + n2, :],
                                in_=gshardB[o - HLOC:o - HLOC + n2, :])
                        nc.tensor.matmul(out=aps[:], lhsT=dg[:nv, :],
                                         rhs=xo_b[:nv, :], start=False,
                                         stop=True)
                    yield b, aps

            # ================= layer 1 (BN1 stats accumulated per block) ==
            for b, aps in agg_layer(1):
                axt = wp.tile([P, P], bf16, tag="axt")
                nc.scalar.activation(out=axt[:], in_=aps[:],
                                     func=mybir.ActivationFunctionType.Copy)
                nv = P if b < NBLK - 1 else LASTV
                for j in range(2):
                    h1p = ps_mm.tile([P, P], f32, tag="mm", space="PSUM")
                    nc.tensor.matmul(out=h1p[:],
                                     lhsT=w1b[:, j * P:(j + 1) * P],
                                     rhs=axt[:], start=True, stop=True)
                    nc.scalar.activation(
                        out=r_st[j][:, b * P:(b + 1) * P], in_=h1p[:],
                        func=mybir.ActivationFunctionType.Relu,
                        bias=b1_sb[:, j:j + 1], scale=1.0)
                    rsl = r_st[j][:, b * P:b * P + nv]
                    nc.vector.reduce_sum(
                        out=s1a[:, j * NBLK + b:j * NBLK + b + 1],
                        in_=rsl, axis=mybir.AxisListType.X)
                    sqt = wp.tile([P, P], f32, tag="sqt")
                    nc.vector.scalar_tensor_tensor(
                        out=sqt[:, :nv], in0=rsl, scalar=0.0, in1=rsl,
                        op0=mybir.AluOpType.add, op1=mybir.AluOpType.mult,
                        accum_out=s2a[:, j * NBLK + b:j * NBLK + b + 1])

            # ---------- BN1 stats (fold per-block partials) ----------
            for j in range(2):
                nc.vector.reduce_sum(out=stats_sb[:, j:j + 1],
                                     in_=s1a[:, j * NBLK:(j + 1) * NBLK],
                                     axis=mybir.AxisListType.X)
                nc.vector.reduce_sum(out=stats_sb[:, 2 + j:3 + j],
                                     in_=s2a[:, j * NBLK:(j + 1) * NBLK],
                                     axis=mybir.AxisListType.X)
            nc.sync.dma_start(out=st1_in[:], in_=stats_sb[:])
            nc.gpsimd.collective_compute(
                "AllReduce", mybir.AluOpType.add,
                replica_groups=[list(range(NC))],
                ins=[st1_in.opt()], outs=[st1_out.opt()])
            st1 = wp.tile([P, 4], f32, tag="st1")
            nc.sync.dma_start(out=st1[:], in_=st1_out[:])
            # mean/var -> scale/shift  [128, 2]
            a1 = wp.tile([P, 2], f32, tag="a1")
            sh1 = wp.tile([P, 2], f32, tag="sh1")
            mean1 = wp.tile([P, 2], f32, tag="mean1")
            var1 = wp.tile([P, 2], f32, tag="var1")
            nc.vector.tensor_scalar(out=mean1[:], in0=st1[:, 0:2],
                                    scalar1=1.0 / N, scalar2=None,
                                    op0=mybir.AluOpType.mult)
            nc.vector.tensor_scalar(out=var1[:], in0=st1[:, 2:4],
                                    scalar1=1.0 / N, scalar2=None,
                                    op0=mybir.AluOpType.mult)
            nc.vector.scalar_tensor_tensor(
                out=a1[:], in0=mean1[:], scalar=0.0, in1=mean1[:],
                op0=mybir.AluOpType.add, op1=mybir.AluOpType.mult)
            nc.vector.tensor_tensor(out=var1[:], in0=var1[:], in1=a1[:],
                                    op=mybir.AluOpType.subtract)
            nc.vector.tensor_scalar(out=var1[:], in0=var1[:], scalar1=EPS,
                                    scalar2=None, op0=mybir.AluOpType.add)
            nc.scalar.activation(out=var1[:], in_=var1[:],
                                 func=mybir.ActivationFunctionType.Sqrt)
            nc.vector.reciprocal(out=var1[:], in_=var1[:])
            nc.vector.tensor_tensor(out=a1[:], in0=var1[:], in1=g1_sb[:],
                                    op=mybir.AluOpType.mult)
            nc.vector.tensor_tensor(out=sh1[:], in0=a1[:], in1=mean1[:],
                                    op=mybir.AluOpType.mult)
            nc.vector.tensor_tensor(out=sh1[:], in0=bt1_sb[:], in1=sh1[:],
                                    op=mybir.AluOpType.subtract)

            # ---------- fold BN1 into W2: W2s = a1*W2 (bf16), shw = sh1@W2 --
            for j in range(2):
                nc.vector.tensor_scalar(
                    out=w2s[:, j * P:(j + 1) * P],
                    in0=w2_sb[:, j * P:(j + 1) * P],
                    scalar1=a1[:, j:j + 1], scalar2=None,
                    op0=mybir.AluOpType.mult)
            shp = ps_tr.tile([P, 2 * P], f32, tag="sps", space="PSUM")
            for j in range(2):
                nc.tensor.matmul(out=shp[0:1, :P],
                                 lhsT=sh1[:, j:j + 1],
                                 rhs=w2_sb[:, j * P:(j + 1) * P],
                                 start=(j == 0), stop=(j == 1))
            nc.vector.tensor_copy(out=shw[:], in_=shp[0:1, :P])

            # ---------- G = relu(H1) @ W2s + ones*shw (bf16), AllGather ----
            for b in range(NBLK):
                gp = ps_mm.tile([P, P], f32, tag="mm", space="PSUM")
                for j in range(2):
                    nc.tensor.matmul(out=gp[:],
                                     lhsT=r_st[j][:, b * P:(b + 1) * P],
                                     rhs=w2s[:, j * P:(j + 1) * P],
                                     start=(j == 0), stop=False)
                nc.tensor.matmul(out=gp[:], lhsT=onesb_sb[:],
                                 rhs=shw[:], start=False, stop=True)
                gsb = wp.tile([P, P], bf16, tag="gsb")
                nc.scalar.activation(out=gsb[:], in_=gp[:],
                                     func=mybir.ActivationFunctionType.Copy)
                nv = P if b < NBLK - 1 else LASTV
                lo, hi = b * P, b * P + nv
                if lo < HLOC:
                    n1 = min(hi, HLOC) - lo
                    nc.sync.dma_start(out=gshardA[lo:lo + n1, :],
                                      in_=gsb[:n1, :])
                if hi > HLOC:
                    o = max(lo, HLOC)
                    n2 = hi - o
                    nc.sync.dma_start(
                        out=gshardB[o - HLOC:o - HLOC + n2, :],
                        in_=gsb[o - lo:o - lo + n2, :])
                if hi >= HLOC and lo < HLOC:
                    # A half complete: start its AllGather while the B-half
                    # G blocks are still being computed
                    nc.gpsimd.collective_compute(
                        "AllGather", mybir.AluOpType.bypass,
                        replica_groups=[list(range(NC))],
                        ins=[gshardA.opt()], outs=[g_tabA.opt()])
            nc.gpsimd.collective_compute(
                "AllGather", mybir.AluOpType.bypass,
                replica_groups=[list(range(NC))],
                ins=[gshardB.opt()], outs=[g_tabB.opt()])

            # ================= layer 2 =================
            for b, aps in agg_layer(2):
                t1 = wp.tile([P, P], f32, tag="t1")
                nc.vector.tensor_tensor(out=t1[:], in0=aps[:], in1=b2b_sb[:],
                                        op=mybir.AluOpType.add)
                nc.scalar.activation(
                    out=h2_st[:, b * P:(b + 1) * P], in_=t1[:],
                    func=mybir.ActivationFunctionType.Relu)
                nv2 = P if b < NBLK - 1 else LASTV
                nc.sync.dma_start(out=h2loc[b * P:b * P + nv2, :],
                                  in_=h2_st[:nv2, b * P:(b + 1) * P])
                # stats: sum + sumsq via ones-matmul (last block: masked ones)
                jsel = 0 if b < NBLK - 1 else 1
                sq = wp.tile([P, P], f32, tag="sq")
                nc.vector.tensor_tensor(out=sq[:],
                                        in0=h2_st[:, b * P:(b + 1) * P],
                                        in1=h2_st[:, b * P:(b + 1) * P],
                                        op=mybir.AluOpType.mult)
                sps1 = ps_tr.tile([P, 2 * P], f32, tag="sps", space="PSUM")
                nc.tensor.matmul(out=sps1[0:1, :P],
                                 lhsT=ones_sb[:, jsel:jsel + 1],
                                 rhs=h2_st[:, b * P:(b + 1) * P],
                                 start=True, stop=True)
                nc.tensor.matmul(out=sps1[0:1, P:],
                                 lhsT=ones_sb[:, jsel:jsel + 1],
                                 rhs=sq[:], start=True, stop=True)
                if b == 0:
                    nc.vector.tensor_copy(out=bn2acc[:, :DEMB],
                                          in_=sps1[0:1, :P])
                    nc.vector.tensor_copy(out=bn2acc[:, DEMB:],
                                          in_=sps1[0:1, P:])
                else:
                    nc.vector.tensor_tensor(out=bn2acc[:, :DEMB],
                                            in0=bn2acc[:, :DEMB],
                                            in1=sps1[0:1, :P],
                                            op=mybir.AluOpType.add)
                    nc.vector.tensor_tensor(out=bn2acc[:, DEMB:],
                                            in0=bn2acc[:, DEMB:],
                                            in1=sps1[0:1, P:],
                                            op=mybir.AluOpType.add)

            # ---------- BN2 stats collective (segmax overlaps the wait) --
            nc.sync.dma_start(out=st2_in[:], in_=bn2acc[:])
            nc.gpsimd.collective_compute(
                "AllReduce", mybir.AluOpType.add,
                replica_groups=[list(range(NC))],
                ins=[st2_in.opt()], outs=[st2_out.opt()])

            # ---------- segment max over RAW h2 (gamma2>0 => affine after) --
            neg = wp.tile([P, P], f32, tag="neg")
            nc.vector.memset(neg[:], -1e38)
            for r in range(5):
                nc.sync.dma_start(out=segtab[r * P:(r + 1) * P, :], in_=neg[:])
            nc.vector.memset(segcols[:], -1e38)
            # consolidated slot gathers: 4 big ping-ponged gathers instead of
            # one tiny gather per slot chunk
            NSEG = 4
            CPG = (TC + NSEG - 1) // NSEG
            seg_tiles = {}

            def seg_gather(g):
                n = min(CPG, TC - g * CPG)
                t = gpool.tile([P, n * P], f32, tag="sbig",
                               name=f"sbig{g}", bufs=2)
                nc.gpsimd.dma_gather(
                    out_ap=t[:, 0:n * P].rearrange("p (c e) -> p c e", e=P),
                    in_ap=h2loc[:],
                    idxs_ap=sel_sb[:, g * CPG * 8:(g * CPG + n) * 8],
                    num_idxs=n * P, num_idxs_reg=n * P, elem_size=DEMB,
                    single_packet=False,
                    queue_num=next_q(),
                )
                return t

            for s in range(SMAX):
                nch = int(scount[s])
                tp = ps_tr.tile([P, 2 * P], f32, tag="sps", space="PSUM")
                for j in range(nch):
                    col = int(sch0[s]) + j
                    g = col // CPG
                    if g not in seg_tiles:
                        seg_tiles[g] = seg_gather(g)
                    t = seg_tiles[g]
                    nc.tensor.transpose(
                        out=tp[:, j * P:(j + 1) * P],
                        in_=t[:, (col - g * CPG) * P:(col - g * CPG + 1) * P],
                        identity=ident[:])
                nc.vector.reduce_max(out=segcols[:, s:s + 1],
                                     in_=tp[:, :nch * P],
                                     axis=mybir.AxisListType.X)

            # ---------- BN2 scale/shift ----------
            st2 = wp.tile([1, 2 * DEMB], f32, tag="st2")
            nc.sync.dma_start(out=st2[:], in_=st2_out[:])
            a2 = wp.tile([1, DEMB], f32, tag="a2")
            sh2 = wp.tile([1, DEMB], f32, tag="sh2")
            mean2 = wp.tile([1, DEMB], f32, tag="mean2")
            var2 = wp.tile([1, DEMB], f32, tag="var2")
            nc.vector.tensor_scalar(out=mean2[:], in0=st2[:, :DEMB],
                                    scalar1=1.0 / N, scalar2=None,
                                    op0=mybir.AluOpType.mult)
            nc.vector.tensor_scalar(out=var2[:], in0=st2[:, DEMB:],
                                    scalar1=1.0 / N, scalar2=None,
                                    op0=mybir.AluOpType.mult)
            nc.vector.scalar_tensor_tensor(
                out=a2[:], in0=mean2[:], scalar=0.0, in1=mean2[:],
                op0=mybir.AluOpType.add, op1=mybir.AluOpType.mult)
            nc.vector.tensor_tensor(out=var2[:], in0=var2[:], in1=a2[:],
                                    op=mybir.AluOpType.subtract)
            nc.vector.tensor_scalar(out=var2[:], in0=var2[:], scalar1=EPS,
                                    scalar2=None, op0=mybir.AluOpType.add)
            nc.scalar.activation(out=var2[:], in_=var2[:],
                                 func=mybir.ActivationFunctionType.Sqrt)
            nc.vector.reciprocal(out=var2[:], in_=var2[:])
            nc.vector.tensor_tensor(out=a2[:], in0=var2[:], in1=g2_sb[:],
                                    op=mybir.AluOpType.mult)
            nc.vector.tensor_tensor(out=sh2[:], in0=a2[:], in1=mean2[:],
                                    op=mybir.AluOpType.mult)
            nc.vector.tensor_tensor(out=sh2[:], in0=bt2_sb[:], in1=sh2[:],
                                    op=mybir.AluOpType.subtract)
            # broadcast a2 / sh2 across partitions via ones-matmul
            bps = ps_tr.tile([P, 2 * P], f32, tag="sps", space="PSUM")
            nc.tensor.matmul(out=bps[:, :P], lhsT=onesr_sb[:],
                             rhs=a2[:], start=True, stop=True)
            nc.tensor.matmul(out=bps[:, P:], lhsT=onesr_sb[:],
                             rhs=sh2[:], start=True, stop=True)
            a2b = stp.tile([P, P], f32)
            nc.vector.tensor_copy(out=a2b[:], in_=bps[:, :P])
            s2b = stp.tile([P, P], f32)
            nc.vector.tensor_copy(out=s2b[:], in_=bps[:, P:])

            # slot maxes -> [slot, feat] rows; apply BN2 affine there
            tp2 = ps_tr.tile([P, 2 * P], f32, tag="sps", space="PSUM")
            nc.tensor.transpose(out=tp2[:, :P], in_=segcols[:],
                                identity=ident[:])
            slotrows = wp.tile([P, P], f32, tag="slotrows")
            nc.vector.tensor_copy(out=slotrows[:], in_=tp2[:, :P])
            nc.vector.tensor_tensor(out=slotrows[:], in0=slotrows[:],
                                    in1=a2b[:], op=mybir.AluOpType.mult)
            nc.vector.tensor_tensor(out=slotrows[:], in0=slotrows[:],
                                    in1=s2b[:], op=mybir.AluOpType.add)
            nc.gpsimd.indirect_dma_start(
                out=segtab[:], out_offset=bass.IndirectOffsetOnAxis(
                    ap=offe_sb[:, :1], axis=0),
                in_=slotrows[:], in_offset=None)
            nc.gpsimd.collective_compute(
                "AllReduce", mybir.AluOpType.max,
                replica_groups=[list(range(NC))],
                ins=[segtab[0:NG, :]], outs=[arout.opt()])
            nc.sync.dma_start(out=out[:], in_=arout[:])

    nc.compile()
    return nc


def _in_maps(x, W1, b1, gamma1, beta1, W2, b2, gamma2, beta2, meta):
    x = np.ascontiguousarray(np.asarray(x, dtype=np.float32))
    iota_b = np.tile(np.arange(P, dtype=np.float32), (P, 1)).astype(
        ml_dtypes.bfloat16)
    onesel = np.ones((P, 2), dtype=np.float32)
    onesel[LASTV:, 1] = 0.0
    assert np.all(np.asarray(gamma2) > 0), "segmax-before-BN2 needs gamma2>0"
    w2 = np.asarray(W2, dtype=np.float32)
    w2p = np.zeros((P, DHID), dtype=np.float32)
    for j in range(2):
        w2p[:, j * P:(j + 1) * P] = w2[j * P:(j + 1) * P, :]
    b2b = np.tile(np.asarray(b2, dtype=np.float32)[None, :], (P, 1))
    common = {
        "x_tab": x,
        "w1": np.asarray(W1, dtype=np.float32),
        "w2p": w2p,
        "b1h": np.asarray(b1, dtype=np.float32).reshape(2, P).T.copy(),
        "b2b": b2b,
        "g1h": np.asarray(gamma1, dtype=np.float32).reshape(2, P).T.copy(),
        "bt1h": np.asarray(beta1, dtype=np.float32).reshape(2, P).T.copy(),
        "g2r": np.asarray(gamma2, dtype=np.float32)[None, :],
        "bt2r": np.asarray(beta2, dtype=np.float32)[None, :],
        "iota_b": iota_b,
        "onesel": onesel,
        "onesrow": np.ones((1, P), dtype=np.float32),
        "onesb": np.ones((1, P), dtype=ml_dtypes.bfloat16),
    }
    maps = []
    for c in range(NC):
        m = dict(common)
        m["x_own"] = x[c * NLOC:(c + 1) * NLOC].astype(ml_dtypes.bfloat16)
        m["src16"] = meta["src_host"][c]
        m["dstlB"] = meta["dstl_host"][c].astype(ml_dtypes.bfloat16)
        m["normA"] = meta["norm_host"][c]
        m["src16b"] = meta["src2_host"][c]
        m["dstlB2"] = meta["dstl2_host"][c].astype(ml_dtypes.bfloat16)
        m["normB2"] = meta["norm2_host"][c].astype(ml_dtypes.bfloat16)
        m["dinv2b"] = meta["dinv2"][c].astype(ml_dtypes.bfloat16)
        m["selidx"] = meta["selidx"][c]
        m["offe"] = meta["offe"][c]
        maps.append(m)
    return maps


def run(inputs, debug=False, trace=False):  # debug kept for test.py compat
    meta = _prep(inputs["x"], inputs["edge_index"], inputs["batch"])
    key = ("v13", meta["totch"], meta["totch2"], meta["SMAX"],
           tuple(meta["cc"].reshape(-1)), tuple(meta["cc2"].reshape(-1)),
           tuple(meta["scount"]))
    if key not in _cache:
        _cache[key] = _build(meta["cc"], meta["totch"], meta["cc2"],
                             meta["totch2"], meta["SMAX"], meta["scount"])
    nc = _cache[key]
    maps = _in_maps(inputs["x"], inputs["W1"], inputs["b1"],
                    inputs["gamma1"], inputs["beta1"], inputs["W2"],
                    inputs["b2"], inputs["gamma2"], inputs["beta2"], meta)
    res = bass_utils.run_bass_kernel_spmd(
        nc, maps, core_ids=list(range(NC)), trace=trace)
    out = np.array(res.results[0]["out"])
    empty = meta["gcounts"] == 0
    if empty.any():
        out[empty] = -np.inf
    return out, res, meta


def kernel(**inputs) -> np.ndarray:
    out, _, _ = run(inputs)
    return out


# revision 20
# speedup vs baseline: 1.0323x; 1.0323x over previous
"""GCN (2-layer, sym-norm + self-loops, BN, segment-max pool) on 8 TRN2 cores.

Strategy (v2):
  - nodes sharded 8x6250 (contiguous); edges sharded by dst owner.
  - layer-1 aggregation reassociated: AX = A_norm @ X (128-dim), then
    H1 = AX @ W1 (per-node transform on local shard).
  - gathers of table rows (X / G) via gpsimd.dma_gather (int16 idx, table
    split in two 25000-row halves); aggregation via one-hot mask matmuls
    into PSUM per 128-dst block; self-loops via diag matmul (no gather).
  - v2: gathers round-robin over 4 SWDGE queues (Q7 core-pair per queue
    pipelines ~2 deep -> ~2x); all mask/matmul work in bf16 (fp32 psum);
    BN1 folded into W2 (G = relu(H1) @ (a1*W2) + sh1@W2), G/AllGather in
    bf16.
  - BN stats + final per-graph max via collectives (AllReduce), G via
    AllGather.
  - program is identical on all 8 cores; all per-core variation is data.
"""
import sys

if "/opt/trn_rl_repo" not in sys.path:
    sys.path.insert(0, "/opt/trn_rl_repo")

import numpy as np
import ml_dtypes

import concourse.bacc as bacc
import concourse.bass as bass
import concourse.tile as tile
from concourse import bass_utils, mybir
from concourse.masks import make_identity

N = 50000
E = 800000
NG = 512
DIN, DHID, DEMB = 128, 256, 128
EPS = 1e-5
NC = 8
NLOC = N // NC          # 6250
NBLK = (NLOC + 127) // 128   # 49
LASTV = NLOC - (NBLK - 1) * 128  # 106 valid rows in last block
HALF = 25000
P = 128
NQ = 4                  # SWDGE queues for gather ping-pong

_cache = {}


def _wrap16(v):
    """[k*128] int16 -> [128, 8*k] in the 16-partition-wrapped, x8-replicated
    layout dma_gather wants (element j -> partition j%16, col j//16)."""
    v = np.asarray(v, dtype=np.int16)
    k = v.size // 16
    w = v.reshape(k, 16).T          # [16, k]
    return np.tile(w, (8, 1))       # [128, k]


def _prep(x, edge_index, batch):
    """Host-side graph preprocessing -> per-core data + program constants."""
    src = np.asarray(edge_index[0], dtype=np.int64)
    dst = np.asarray(edge_index[1], dtype=np.int64)
    batch = np.asarray(batch, dtype=np.int64)

    deg = np.bincount(dst, minlength=N).astype(np.float64) + 1.0
    dinv = 1.0 / np.sqrt(deg)
    norm = (dinv[src] * dinv[dst]).astype(np.float32)

    core = dst // NLOC
    dloc = dst % NLOC
    blk = dloc // P
    dstl = (dloc % P).astype(np.float32)
    half = (src >= HALF).astype(np.int64)
    srel = (src - HALF * half).astype(np.int64)

    # group id: (core, blk, half); sort edges into groups
    gid = (core * NBLK + blk) * 2 + half
    order = np.argsort(gid, kind="stable")
    gid_s = gid[order]
    counts = np.bincount(gid, minlength=NC * NBLK * 2).reshape(NC, NBLK, 2)
    # chunks per (blk, half): max over cores, in units of 128 edges
    cc = (counts.max(axis=0) + P - 1) // P        # [NBLK, 2]
    totch = int(cc.sum())                          # chunks per core (uniform)
    TOT = totch * P

    # per-group slot offsets within the padded per-core layout
    goff = np.zeros((NBLK, 2), dtype=np.int64)
    run = 0
    for b in range(NBLK):
        for h in range(2):
            goff[b, h] = run
            run += int(cc[b, h]) * P

    srel_s = srel[order]
    dstl_s = dstl[order]
    norm_s = norm[order]
    core_s = core[order]
    blk_s = blk[order]
    half_s = half[order]
    # rank within group
    grp_start = np.zeros(NC * NBLK * 2, dtype=np.int64)
    np.cumsum(counts.reshape(-1)[:-1], out=grp_start[1:])
    rank = np.arange(E, dtype=np.int64) - grp_start[gid_s]

    slot = goff[blk_s, half_s] + rank              # slot within its core
    src16 = np.zeros((NC, TOT), dtype=np.int16)
    dstlA = np.zeros((NC, TOT), dtype=np.float32)
    normA = np.zeros((NC, TOT), dtype=np.float32)
    src16[core_s, slot] = srel_s.astype(np.int16)
    dstlA[core_s, slot] = dstl_s
    normA[core_s, slot] = norm_s

    # device layouts
    dstl_host = dstlA.reshape(NC, totch, P).transpose(0, 2, 1).copy()   # [NC,128,totch]
    norm_host = normA.reshape(NC, totch, P).transpose(0, 2, 1).copy()
    src_host = np.zeros((NC, P, 8 * totch), dtype=np.int16)
    ch0 = np.concatenate([[0], np.cumsum(cc.reshape(-1))]).astype(np.int64)
    for g in range(NBLK * 2):
        c0, c1 = int(ch0[g]), int(ch0[g + 1])
        if c0 == c1:
            continue
        for c in range(NC):
            seg = src16[c, c0 * P:c1 * P]
            src_host[c, :, c0 * 8:c1 * 8] = _wrap16(seg)

    # self-loop diag values 1/deg per local block
    dinv2 = np.zeros((NC, P, NBLK), dtype=np.float32)
    for c in range(NC):
        v = (1.0 / deg[c * NLOC:(c + 1) * NLOC]).astype(np.float32)
        pad = np.zeros(NBLK * P, dtype=np.float32)
        pad[:NLOC] = v
        dinv2[c] = pad.reshape(NBLK, P).T

    # ---- segment-max slots ----
    gcounts = np.bincount(batch, minlength=NG)
    gstart = np.concatenate([[0], np.cumsum(gcounts)]).astype(np.int64)
    slots = [[] for _ in range(NC)]               # (graph, lo_local, hi_local)
    for g in range(NG):
        a, b2 = int(gstart[g]), int(gstart[g + 1])
        if a == b2:
            continue
        c0, c1 = a // NLOC, (b2 - 1) // NLOC
        for c in range(c0, c1 + 1):
            lo = max(a, c * NLOC) - c * NLOC
            hi = min(b2, (c + 1) * NLOC) - c * NLOC
            assert hi - lo <= 2 * P, "graph piece exceeds 256 nodes"
            slots[c].append((g, lo, hi))
    SMAX = max(len(s) for s in slots)
    scount = np.ones(SMAX, dtype=np.int64)
    for c in range(NC):
        for s, (g, lo, hi) in enumerate(slots[c]):
            scount[s] = max(scount[s], (hi - lo + P - 1) // P)
    sch0 = np.concatenate([[0], np.cumsum(scount)]).astype(np.int64)
    TC = int(scount.sum())
    selidx = np.zeros((NC, P, 8 * TC), dtype=np.int16)
    offe = np.zeros((NC, P, 1), dtype=np.int32)
    for c in range(NC):
        for s in range(P):
            offe[c, s, 0] = 512 + s
        for s in range(SMAX):
            g, lo, hi = slots[c][s] if s < len(slots[c]) else (None, 0, 1)
            if g is not None:
                offe[c, s, 0] = g
            for j in range(int(scount[s])):
                a = lo + j * P
                idx = np.full(P, lo, dtype=np.int16)
                n = max(0, min(hi - a, P))
                if n > 0:
                    idx[:n] = np.arange(a, a + n, dtype=np.int16)
                col = int(sch0[s]) + j
                selidx[c, :, 8 * col:8 * (col + 1)] = _wrap16(idx)

    # ---- second edge grouping for layer 2 (tables split at HLOC within
    # each owner shard so the two AllGather halves can be separate) ----
    HLOC = NLOC // 2
    srcc = src // NLOC
    srcr = src % NLOC
    half2 = (srcr >= HLOC).astype(np.int64)
    srel2 = (srcc * HLOC + srcr - HLOC * half2).astype(np.int64)
    gid2 = (core * NBLK + blk) * 2 + half2
    order2 = np.argsort(gid2, kind="stable")
    gid2_s = gid2[order2]
    counts2 = np.bincount(gid2, minlength=NC * NBLK * 2).reshape(NC, NBLK, 2)
    cc2 = (counts2.max(axis=0) + P - 1) // P
    totch2 = int(cc2.sum())
    TOT2 = totch2 * P
    goff2 = np.zeros((NBLK, 2), dtype=np.int64)
    run2 = 0
    for b in range(NBLK):
        for h in range(2):
            goff2[b, h] = run2
            run2 += int(cc2[b, h]) * P
    grp2_start = np.zeros(NC * NBLK * 2, dtype=np.int64)
    np.cumsum(counts2.reshape(-1)[:-1], out=grp2_start[1:])
    rank2 = np.arange(E, dtype=np.int64) - grp2_start[gid2_s]
    slot2 = goff2[blk_s2 := blk[order2], half2[order2]] + rank2
    core_s2 = core[order2]
    src16_2 = np.zeros((NC, TOT2), dtype=np.int16)
    dstl2A = np.zeros((NC, TOT2), dtype=np.float32)
    norm2A = np.zeros((NC, TOT2), dtype=np.float32)
    src16_2[core_s2, slot2] = srel2[order2].astype(np.int16)
    dstl2A[core_s2, slot2] = dstl[order2]
    norm2A[core_s2, slot2] = norm[order2]
    dstl2_host = dstl2A.reshape(NC, totch2, P).transpose(0, 2, 1).copy()
    norm2_host = norm2A.reshape(NC, totch2, P).transpose(0, 2, 1).copy()
    src2_host = np.zeros((NC, P, 8 * totch2), dtype=np.int16)
    ch02 = np.concatenate([[0], np.cumsum(cc2.reshape(-1))]).astype(np.int64)
    for g in range(NBLK * 2):
        c0, c1 = int(ch02[g]), int(ch02[g + 1])
        if c0 == c1:
            continue
        for c in range(NC):
            seg = src16_2[c, c0 * P:c1 * P]
            src2_host[c, :, c0 * 8:c1 * 8] = _wrap16(seg)

    meta = {
        "cc": cc, "totch": totch, "SMAX": SMAX, "scount": scount,
        "src_host": src_host, "dstl_host": dstl_host, "norm_host": norm_host,
        "cc2": cc2, "totch2": totch2, "src2_host": src2_host,
        "dstl2_host": dstl2_host, "norm2_host": norm2_host,
        "dinv2": dinv2, "selidx": selidx, "offe": offe,
        "gcounts": gcounts,
    }
    return meta


def _build(cc, totch, cc2, totch2, SMAX, scount):
    """Build the SPMD program. cc: [NBLK,2] chunk counts, identical across
    cores."""
    TC = int(scount.sum())
    sch0 = np.concatenate([[0], np.cumsum(scount)]).astype(np.int64)
    nc = bacc.Bacc("TRN2", target_bir_lowering=False, debug=False,
                   enable_asserts=True, num_devices=NC, num_swdge_queues=NQ)
    f32, bf16, i16, i32 = (mybir.dt.float32, mybir.dt.bfloat16,
                           mybir.dt.int16, mybir.dt.int32)

    def din(name, shape, dt=f32):
        return nc.dram_tensor(name, shape, dt, kind="ExternalInput").ap()

    x_tab = din("x_tab", [N, DIN])
    x_own = din("x_own", [NLOC, DIN], bf16)
    w1 = din("w1", [DIN, DHID])
    w2p = din("w2p", [P, DHID])            # w2p[p, j*128+c] = W2[j*128+p, c]
    b1h = din("b1h", [P, 2])
    b2b = din("b2b", [P, DEMB])
    g1h = din("g1h", [P, 2])
    bt1h = din("bt1h", [P, 2])
    g2r = din("g2r", [1, DEMB])
    bt2r = din("bt2r", [1, DEMB])
    iota_b = din("iota_b", [P, P], bf16)
    onesel = din("onesel", [P, 2])
    onesrow = din("onesrow", [1, P])
    onesb = din("onesb", [1, P], bf16)
    src16 = din("src16", [P, 8 * totch], i16)
    dstlB = din("dstlB", [P, totch], bf16)
    normA = din("normA", [P, totch])
    src16b = din("src16b", [P, 8 * totch2], i16)
    dstlB2 = din("dstlB2", [P, totch2], bf16)
    normB2 = din("normB2", [P, totch2], bf16)
    dinv2b = din("dinv2b", [P, NBLK], bf16)
    selidx = din("selidx", [P, 8 * TC], i16)
    offe = din("offe", [P, 1], i32)

    out = nc.dram_tensor("out", [NG, DEMB], f32, kind="ExternalOutput").ap()

    # chunk column offsets per (blk, half)
    ch0 = np.concatenate([[0], np.cumsum(cc.reshape(-1))]).astype(np.int64)
    ch02 = np.concatenate([[0], np.cumsum(cc2.reshape(-1))]).astype(np.int64)
    HLOC = NLOC // 2

    qctr = [0]

    def next_q():
        q = qctr[0] % NQ
        qctr[0] += 1
        return q

    with tile.TileContext(nc) as tc:
        with (
            tc.tile_pool(name="const", bufs=1) as cp,
            tc.tile_pool(name="meta", bufs=1) as mp,
            tc.tile_pool(name="store", bufs=1) as stp,
            tc.tile_pool(name="gath", bufs=3) as gpool,
            tc.tile_pool(name="gathb", bufs=2) as gbp,
            tc.tile_pool(name="mask", bufs=2) as mkp,
            tc.tile_pool(name="work", bufs=3) as wp,
            tc.tile_pool(name="ps_agg", bufs=2, space="PSUM") as ps_agg,
            tc.tile_pool(name="ps_mm", bufs=2, space="PSUM") as ps_mm,
            tc.tile_pool(name="ps_tr", bufs=2, space="PSUM") as ps_tr,
            tc.tile_pool(name="dram", bufs=1, space="DRAM") as dr,
        ):
            # ---------- constants / meta ----------
            src_sb = mp.tile([P, 8 * totch], i16)
            nc.sync.dma_start(out=src_sb[:], in_=src16[:])
            dstl_sb = mp.tile([P, totch], bf16)
            nc.sync.dma_start(out=dstl_sb[:], in_=dstlB[:])
            norm_sb = mp.tile([P, totch], f32)
            nc.sync.dma_start(out=norm_sb[:], in_=normA[:])
            src2_sb = mp.tile([P, 8 * totch2], i16)
            nc.sync.dma_start(out=src2_sb[:], in_=src16b[:])
            dstl2_sb = mp.tile([P, totch2], bf16)
            nc.sync.dma_start(out=dstl2_sb[:], in_=dstlB2[:])
            norm2b_sb = mp.tile([P, totch2], bf16)
            nc.sync.dma_start(out=norm2b_sb[:], in_=normB2[:])
            dinv2_sb = mp.tile([P, NBLK], bf16)
            nc.sync.dma_start(out=dinv2_sb[:], in_=dinv2b[:])
            sel_sb = mp.tile([P, 8 * TC], i16)
            nc.sync.dma_start(out=sel_sb[:], in_=selidx[:])
            offe_sb = mp.tile([P, 1], i32)
            nc.sync.dma_start(out=offe_sb[:], in_=offe[:])

            w1f = cp.tile([P, DHID], f32)
            nc.sync.dma_start(out=w1f[:], in_=w1[:])
            w1b = cp.tile([P, DHID], bf16)
            nc.vector.tensor_copy(out=w1b[:], in_=w1f[:])
            w2_sb = cp.tile([P, DHID], f32)
            nc.sync.dma_start(out=w2_sb[:], in_=w2p[:])
            b1_sb = cp.tile([P, 2], f32)
            nc.sync.dma_start(out=b1_sb[:], in_=b1h[:])
            b2b_sb = cp.tile([P, DEMB], f32)
            nc.sync.dma_start(out=b2b_sb[:], in_=b2b[:])
            g1_sb = cp.tile([P, 2], f32)
            nc.sync.dma_start(out=g1_sb[:], in_=g1h[:])
            bt1_sb = cp.tile([P, 2], f32)
            nc.sync.dma_start(out=bt1_sb[:], in_=bt1h[:])
            g2_sb = cp.tile([1, DEMB], f32)
            nc.sync.dma_start(out=g2_sb[:], in_=g2r[:])
            bt2_sb = cp.tile([1, DEMB], f32)
            nc.sync.dma_start(out=bt2_sb[:], in_=bt2r[:])
            iotab_sb = cp.tile([P, P], bf16)
            nc.sync.dma_start(out=iotab_sb[:], in_=iota_b[:])
            ones_sb = cp.tile([P, 2], f32)
            nc.sync.dma_start(out=ones_sb[:], in_=onesel[:])
            onesr_sb = cp.tile([1, P], f32)
            nc.sync.dma_start(out=onesr_sb[:], in_=onesrow[:])
            onesb_sb = cp.tile([1, P], bf16)
            nc.sync.dma_start(out=onesb_sb[:], in_=onesb[:])
            ident = cp.tile([P, P], f32)
            make_identity(nc, ident[:])
            ident_b = cp.tile([P, P], bf16)
            nc.vector.tensor_copy(out=ident_b[:], in_=ident[:])

            # ---------- persistent stores ----------
            s1a = stp.tile([P, 2 * NBLK], f32, tag="s1a")
            s2a = stp.tile([P, 2 * NBLK], f32, tag="s2a")
            r_st0 = stp.tile([P, NBLK * P], bf16, tag="r0")
            r_st1 = stp.tile([P, NBLK * P], bf16, tag="r1")
            r_st = [r_st0, r_st1]
            h2_st = stp.tile([P, NBLK * P], f32)
            stats_sb = stp.tile([P, 4], f32)
            bn2acc = stp.tile([1, 2 * DEMB], f32)
            segcols = stp.tile([P, P], f32)
            w2s = stp.tile([P, DHID], bf16)
            shw = stp.tile([1, P], bf16)

            # ---------- DRAM internals ----------
            gshardA = dr.tile([NLOC // 2, DEMB], bf16)
            gshardB = dr.tile([NLOC // 2, DEMB], bf16)
            g_tabA = dr.tile([N // 2, DEMB], bf16)
            g_tabB = dr.tile([N // 2, DEMB], bf16)
            st1_in = dr.tile([P, 4], f32)
            st1_out = dr.tile([P, 4], f32)
            st2_in = dr.tile([1, 2 * DEMB], f32)
            st2_out = dr.tile([1, 2 * DEMB], f32)
            segtab = dr.tile([640, DEMB], f32)
            arout = dr.tile([NG, DEMB], f32)
            h2loc = dr.tile([NLOC, DEMB], f32)

            def agg_layer(layer):
                """Aggregation for one layer. layer 1: psum AXT [f,d]; layer
                2: psum AG [d,f] node-major. Both use bf16 operands."""
                lcc = cc if layer == 1 else cc2
                lch0 = ch0 if layer == 1 else ch02
                lsrc = src_sb if layer == 1 else src2_sb
                ldstl = dstl_sb if layer == 1 else dstl2_sb

                def gatherh(gt_t, b, h):
                    ng = int(lcc[b, h])
                    if ng == 0:
                        return
                    c0 = int(lch0[2 * b + h])
                    crel = c0 - int(lch0[2 * b])
                    if layer == 1:
                        base = 0 if h == 0 else HALF
                        tab = x_tab[base:base + HALF, :]
                    else:
                        tab = (g_tabA if h == 0 else g_tabB)[:]
                    nc.gpsimd.dma_gather(
                        out_ap=gt_t[:, crel * P:(crel + ng) * P].rearrange(
                            "p (c e) -> p c e", e=P),
                        in_ap=tab,
                        idxs_ap=lsrc[:, c0 * 8:(c0 + ng) * 8],
                        num_idxs=ng * P,
                        num_idxs_reg=ng * P,
                        elem_size=P,
                        single_packet=False,
                        queue_num=next_q(),
                    )

                # layer 2: emit table-A gathers K blocks ahead so the second
                # AllGather drains underneath them
                K = 3 if layer == 2 else 0
                pend = {}

                def mk_tile(b):
                    nch_t = int(lcc[b, 0] + lcc[b, 1])
                    if layer == 1:
                        return gpool.tile([P, nch_t * P], f32, tag="gt",
                                          name=f"gt_{b}", bufs=3)
                    return gpool.tile([P, nch_t * P], bf16, tag="gt2",
                                      name=f"gt2_{b}", bufs=K + 2)

                for bb in range(min(K, NBLK)):
                    pend[bb] = mk_tile(bb)
                    gatherh(pend[bb], bb, 0)

                for b in range(NBLK):
                    nch = int(lcc[b, 0] + lcc[b, 1])
                    if layer == 1:
                        gt = mk_tile(b)
                        gatherh(gt, b, 0)
                        gatherh(gt, b, 1)
                    else:
                        gt = pend.pop(b)
                        gatherh(gt, b, 1)
                        if b + K < NBLK:
                            pend[b + K] = mk_tile(b + K)
                            gatherh(pend[b + K], b + K, 0)
                    # batched bf16 mask build + per-edge norm fold
                    c0b = int(lch0[2 * b])
                    mk = mkp.tile([P, nch * P], bf16, tag="mk")
                    iap = iotab_sb[:]
                    iota_rep = bass.AP(iap.tensor, iap.offset,
                                       [iap.ap[0], [0, nch], [1, P]])
                    nc.vector.tensor_tensor(
                        out=mk[:].rearrange("p (c e) -> p c e", e=P),
                        in0=iota_rep,
                        in1=ldstl[:, c0b:c0b + nch].to_broadcast([P, nch, P]),
                        op=mybir.AluOpType.is_equal)
                    if layer == 1:
                        gtb = gbp.tile([P, nch * P], bf16, tag="gtb")
                        nc.vector.tensor_tensor(
                            out=gtb[:].rearrange("p (c e) -> p c e", e=P),
                            in0=gt[:].rearrange("p (c e) -> p c e", e=P),
                            in1=norm_sb[:, c0b:c0b + nch].to_broadcast(
                                [P, nch, P]),
                            op=mybir.AluOpType.mult)
                    else:
                        gtb = gt
                        nc.vector.tensor_tensor(
                            out=gtb[:].rearrange("p (c e) -> p c e", e=P),
                            in0=gtb[:].rearrange("p (c e) -> p c e", e=P),
                            in1=norm2b_sb[:, c0b:c0b + nch].to_broadcast(
                                [P, nch, P]),
                            op=mybir.AluOpType.mult)
                    aps = ps_agg.tile([P, P], f32, tag="agg", space="PSUM")
                    for j in range(nch):
                        if layer == 1:
                            nc.tensor.matmul(
                                out=aps[:], lhsT=gtb[:, j * P:(j + 1) * P],
                                rhs=mk[:, j * P:(j + 1) * P],
                                start=(j == 0), stop=False)
                        else:
                            nc.tensor.matmul(
                                out=aps[:], lhsT=mk[:, j * P:(j + 1) * P],
                                rhs=gtb[:, j * P:(j + 1) * P],
                                start=(j == 0), stop=False)
                    # self-loop: diag(1/deg) @ own rows (bf16)
                    dg = mkp.tile([P, P], bf16, tag="dg")
                    nc.vector.tensor_tensor(
                        out=dg[:], in0=ident_b[:],
                        in1=dinv2_sb[:, b:b + 1].to_broadcast([P, P]),
                        op=mybir.AluOpType.mult)
                    nv = P if b < NBLK - 1 else LASTV
                    if layer == 1:
                        xo_b = wp.tile([P, P], bf16, tag="xob")
                        nc.sync.dma_start(out=xo_b[:nv, :],
                                          in_=x_own[b * P:b * P + nv, :])
                        nc.tensor.matmul(out=aps[:], lhsT=xo_b[:nv, :],
                                         rhs=dg[:nv, :], start=False,
                                         stop=True)
                    else:
                        xo_b = wp.tile([P, P], bf16, tag="xob")
                        lo, hi = b * P, b * P + nv
                        if lo < HLOC:
                            n1 = min(hi, HLOC) - lo
                            nc.sync.dma_start(out=xo_b[:n1, :],
                                              in_=gshardA[lo:lo + n1, :])
                        if hi > HLOC:
                            o = max(lo, HLOC)
                            n2 = hi - o
                            nc.sync.dma_start(
                                out=xo_b[o - lo:o - lo + n2, :],
                                in_=gshardB[o - HLOC:o - HLOC + n2, :])
                        nc.tensor.matmul(out=aps[:], lhsT=dg[:nv, :],
                                         rhs=xo_b[:nv, :], start=False,
                                         stop=True)
                    yield b, aps

            # ================= layer 1 (BN1 stats accumulated per block) ==
            for b, aps in agg_layer(1):
                axt = wp.tile([P, P], bf16, tag="axt")
                nc.scalar.activation(out=axt[:], in_=aps[:],
                                     func=mybir.ActivationFunctionType.Copy)
                nv = P if b < NBLK - 1 else LASTV
                for j in range(2):
                    h1p = ps_mm.tile([P, P], f32, tag="mm", space="PSUM")
                    nc.tensor.matmul(out=h1p[:],
                                     lhsT=w1b[:, j * P:(j + 1) * P],
                                     rhs=axt[:], start=True, stop=True)
                    nc.scalar.activation(
                        out=r_st[j][:, b * P:(b + 1) * P], in_=h1p[:],
                        func=mybir.ActivationFunctionType.Relu,
                        bias=b1_sb[:, j:j + 1], scale=1.0)
                    rsl = r_st[j][:, b * P:b * P + nv]
                    nc.vector.reduce_sum(
                        out=s1a[:, j * NBLK + b:j * NBLK + b + 1],
                        in_=rsl, axis=mybir.AxisListType.X)
                    sqt = wp.tile([P, P], f32, tag="sqt")
                    nc.vector.scalar_tensor_tensor(
                        out=sqt[:, :nv], in0=rsl, scalar=0.0, in1=rsl,
                        op0=mybir.AluOpType.add, op1=mybir.AluOpType.mult,
                        accum_out=s2a[:, j * NBLK + b:j * NBLK + b + 1])

            # ---------- BN1 stats (fold per-block partials) ----------
            for j in range(2):
                nc.vector.reduce_sum(out=stats_sb[:, j:j + 1],
                                     in_=s1a[:, j * NBLK:(j + 1) * NBLK],
                                     axis=mybir.AxisListType.X)
                nc.vector.reduce_sum(out=stats_sb[:, 2 + j:3 + j],
                                     in_=s2a[:, j * NBLK:(j + 1) * NBLK],
                                     axis=mybir.AxisListType.X)
            nc.sync.dma_start(out=st1_in[:], in_=stats_sb[:])
            nc.gpsimd.collective_compute(
                "AllReduce", mybir.AluOpType.add,
                replica_groups=[list(range(NC))],
                ins=[st1_in.opt()], outs=[st1_out.opt()])
            st1 = wp.tile([P, 4], f32, tag="st1")
            nc.sync.dma_start(out=st1[:], in_=st1_out[:])
            # mean/var -> scale/shift  [128, 2]
            a1 = wp.tile([P, 2], f32, tag="a1")
            sh1 = wp.tile([P, 2], f32, tag="sh1")
            mean1 = wp.tile([P, 2], f32, tag="mean1")
            var1 = wp.tile([P, 2], f32, tag="var1")
            nc.vector.tensor_scalar(out=mean1[:], in0=st1[:, 0:2],
                                    scalar1=1.0 / N, scalar2=None,
                                    op0=mybir.AluOpType.mult)
            nc.vector.tensor_scalar(out=var1[:], in0=st1[:, 2:4],
                                    scalar1=1.0 / N, scalar2=None,
                                    op0=mybir.AluOpType.mult)
            nc.vector.scalar_tensor_tensor(
                out=a1[:], in0=mean1[:], scalar=0.0, in1=mean1[:],
                op0=mybir.AluOpType.add, op1=mybir.AluOpType.mult)
            nc.vector.tensor_tensor(out=var1[:], in0=var1[:], in1=a1[:],
                                    op=mybir.AluOpType.subtract)
            nc.vector.tensor_scalar(out=var1[:], in0=var1[:], scalar1=EPS,
                                    scalar2=None, op0=mybir.AluOpType.add)
            nc.scalar.activation(out=var1[:], in_=var1[:],
                                 func=mybir.ActivationFunctionType.Sqrt)
            nc.vector.reciprocal(out=var1[:], in_=var1[:])
            nc.vector.tensor_tensor(out=a1[:], in0=var1[:], in1=g1_sb[:],
                                    op=mybir.AluOpType.mult)
            nc.vector.tensor_tensor(out=sh1[:], in0=a1[:], in1=mean1[:],
                                    op=mybir.AluOpType.mult)
            nc.vector.tensor_tensor(out=sh1[:], in0=bt1_sb[:], in1=sh1[:],
                                    op=mybir.AluOpType.subtract)

            # ---------- fold BN1 into W2: W2s = a1*W2 (bf16), shw = sh1@W2 --
            for j in range(2):
                nc.vector.tensor_scalar(
                    out=w2s[:, j * P:(j + 1) * P],
                    in0=w2_sb[:, j * P:(j + 1) * P],
                    scalar1=a1[:, j:j + 1], scalar2=None,
                    op0=mybir.AluOpType.mult)
            shp = ps_tr.tile([P, 2 * P], f32, tag="sps", space="PSUM")
            for j in range(2):
                nc.tensor.matmul(out=shp[0:1, :P],
                                 lhsT=sh1[:, j:j + 1],
                                 rhs=w2_sb[:, j * P:(j + 1) * P],
                                 start=(j == 0), stop=(j == 1))
            nc.vector.tensor_copy(out=shw[:], in_=shp[0:1, :P])

            # ---------- G = relu(H1) @ W2s + ones*shw (bf16), AllGather ----
            for b in range(NBLK):
                gp = ps_mm.tile([P, P], f32, tag="mm", space="PSUM")
                for j in range(2):
                    nc.tensor.matmul(out=gp[:],
                                     lhsT=r_st[j][:, b * P:(b + 1) * P],
                                     rhs=w2s[:, j * P:(j + 1) * P],
                                     start=(j == 0), stop=False)
                nc.tensor.matmul(out=gp[:], lhsT=onesb_sb[:],
                                 rhs=shw[:], start=False, stop=True)
                gsb = wp.tile([P, P], bf16, tag="gsb")
                nc.scalar.activation(out=gsb[:], in_=gp[:],
                                     func=mybir.ActivationFunctionType.Copy)
                nv = P if b < NBLK - 1 else LASTV
                lo, hi = b * P, b * P + nv
                if lo < HLOC:
                    n1 = min(hi, HLOC) - lo
                    nc.sync.dma_start(out=gshardA[lo:lo + n1, :],
                                      in_=gsb[:n1, :])
                if hi > HLOC:
                    o = max(lo, HLOC)
                    n2 = hi - o
                    nc.sync.dma_start(
                        out=gshardB[o - HLOC:o - HLOC + n2, :],
                        in_=gsb[o - lo:o - lo + n2, :])
                if hi >= HLOC and lo < HLOC:
                    # A half complete: start its AllGather while the B-half
                    # G blocks are still being computed
                    nc.gpsimd.collective_compute(
                        "AllGather", mybir.AluOpType.bypass,
                        replica_groups=[list(range(NC))],
                        ins=[gshardA.opt()], outs=[g_tabA.opt()])
            nc.gpsimd.collective_compute(
                "AllGather", mybir.AluOpType.bypass,
                replica_groups=[list(range(NC))],
                ins=[gshardB.opt()], outs=[g_tabB.opt()])

            # ================= layer 2 =================
            for b, aps in agg_layer(2):
                t1 = wp.tile([P, P], f32, tag="t1")
                nc.vector.tensor_tensor(out=t1[:], in0=aps[:], in1=b2b_sb[:],
                                        op=mybir.AluOpType.add)
                nc.scalar.activation(
                    out=h2_st[:, b * P:(b + 1) * P], in_=t1[:],
                    func=mybir.ActivationFunctionType.Relu)
                nv2 = P if b < NBLK - 1 else LASTV
                nc.sync.dma_start(out=h2loc[b * P:b * P + nv2, :],
                                  in_=h2_st[:nv2, b * P:(b + 1) * P])
                # stats: sum + sumsq via ones-matmul (last block: masked ones)
                jsel = 0 if b < NBLK - 1 else 1
                sq = wp.tile([P, P], f32, tag="sq")
                nc.vector.tensor_tensor(out=sq[:],
                                        in0=h2_st[:, b * P:(b + 1) * P],
                                        in1=h2_st[:, b * P:(b + 1) * P],
                                        op=mybir.AluOpType.mult)
                sps1 = ps_tr.tile([P, 2 * P], f32, tag="sps", space="PSUM")
                nc.tensor.matmul(out=sps1[0:1, :P],
                                 lhsT=ones_sb[:, jsel:jsel + 1],
                                 rhs=h2_st[:, b * P:(b + 1) * P],
                                 start=True, stop=True)
                nc.tensor.matmul(out=sps1[0:1, P:],
                                 lhsT=ones_sb[:, jsel:jsel + 1],
                                 rhs=sq[:], start=True, stop=True)
                if b == 0:
                    nc.vector.tensor_copy(out=bn2acc[:, :DEMB],
                                          in_=sps1[0:1, :P])
                    nc.vector.tensor_copy(out=bn2acc[:, DEMB:],
                                          in_=sps1[0:1, P:])
                else:
                    nc.vector.tensor_tensor(out=bn2acc[:, :DEMB],
                                            in0=bn2acc[:, :DEMB],
                                            in1=sps1[0:1, :P],
                                            op=mybir.AluOpType.add)
                    nc.vector.tensor_tensor(out=bn2acc[:, DEMB:],
                                            in0=bn2acc[:, DEMB:],
                                            in1=sps1[0:1, P:],
                                            op=mybir.AluOpType.add)

            # ---------- BN2 stats collective (segmax overlaps the wait) --
            nc.sync.dma_start(out=st2_in[:], in_=bn2acc[:])
            nc.gpsimd.collective_compute(
                "AllReduce", mybir.AluOpType.add,
                replica_groups=[list(range(NC))],
                ins=[st2_in.opt()], outs=[st2_out.opt()])

            # ---------- segment max over RAW h2 (gamma2>0 => affine after) --
            neg = wp.tile([P, P], f32, tag="neg")
            nc.vector.memset(neg[:], -1e38)
            for r in range(5):
                nc.sync.dma_start(out=segtab[r * P:(r + 1) * P, :], in_=neg[:])
            nc.vector.memset(segcols[:], -1e38)
            # consolidated slot gathers: 4 big ping-ponged gathers instead of
            # one tiny gather per slot chunk
            NSEG = 4
            CPG = (TC + NSEG - 1) // NSEG
            seg_tiles = {}

            def seg_gather(g):
                n = min(CPG, TC - g * CPG)
                t = gpool.tile([P, n * P], f32, tag="sbig",
                               name=f"sbig{g}", bufs=2)
                nc.gpsimd.dma_gather(
                    out_ap=t[:, 0:n * P].rearrange("p (c e) -> p c e", e=P),
                    in_ap=h2loc[:],
                    idxs_ap=sel_sb[:, g * CPG * 8:(g * CPG + n) * 8],
                    num_idxs=n * P, num_idxs_reg=n * P, elem_size=DEMB,
                    single_packet=False,
                    queue_num=next_q(),
                )
                return t

            for s in range(SMAX):
                nch = int(scount[s])
                tp = ps_tr.tile([P, 2 * P], f32, tag="sps", space="PSUM")
                for j in range(nch):
                    col = int(sch0[s]) + j
                    g = col // CPG
                    if g not in seg_tiles:
                        seg_tiles[g] = seg_gather(g)
                    t = seg_tiles[g]
                    nc.tensor.transpose(
                        out=tp[:, j * P:(j + 1) * P],
                        in_=t[:, (col - g * CPG) * P:(col - g * CPG + 1) * P],
                        identity=ident[:])
                nc.vector.reduce_max(out=segcols[:, s:s + 1],
                                     in_=tp[:, :nch * P],
                                     axis=mybir.AxisListType.X)

            # ---------- BN2 scale/shift ----------
            st2 = wp.tile([1, 2 * DEMB], f32, tag="st2")
            nc.sync.dma_start(out=st2[:], in_=st2_out[:])
            a2 = wp.tile([1, DEMB], f32, tag="a2")
            sh2 = wp.tile([1, DEMB], f32, tag="sh2")
            mean2 = wp.tile([1, DEMB], f32, tag="mean2")
            var2 = wp.tile([1, DEMB], f32, tag="var2")
            nc.vector.tensor_scalar(out=mean2[:], in0=st2[:, :DEMB],
                                    scalar1=1.0 / N, scalar2=None,
                                    op0=mybir.AluOpType.mult)
            nc.vector.tensor_scalar(out=var2[:], in0=st2[:, DEMB:],
                                    scalar1=1.0 / N, scalar2=None,
                                    op0=mybir.AluOpType.mult)
            nc.vector.scalar_tensor_tensor(
                out=a2[:], in0=mean2[:], scalar=0.0, in1=mean2[:],
                op0=mybir.AluOpType.add, op1=mybir.AluOpType.mult)
            nc.vector.tensor_tensor(out=var2[:], in0=var2[:], in1=a2[:],
                                    op=mybir.AluOpType.subtract)
            nc.vector.tensor_scalar(out=var2[:], in0=var2[:], scalar1=EPS,
                                    scalar2=None, op0=mybir.AluOpType.add)
            nc.scalar.activation(out=var2[:], in_=var2[:],
                                 func=mybir.ActivationFunctionType.Sqrt)
            nc.vector.reciprocal(out=var2[:], in_=var2[:])
            nc.vector.tensor_tensor(out=a2[:], in0=var2[:], in1=g2_sb[:],
                                    op=mybir.AluOpType.mult)
            nc.vector.tensor_tensor(out=sh2[:], in0=a2[:], in1=mean2[:],
                                    op=mybir.AluOpType.mult)
            nc.vector.tensor_tensor(out=sh2[:], in0=bt2_sb[:], in1=sh2[:],
                                    op=mybir.AluOpType.subtract)
            # broadcast a2 / sh2 across partitions via ones-matmul
            bps = ps_tr.tile([P, 2 * P], f32, tag="sps", space="PSUM")
            nc.tensor.matmul(out=bps[:, :P], lhsT=onesr_sb[:],
                             rhs=a2[:], start=True, stop=True)
            nc.tensor.matmul(out=bps[:, P:], lhsT=onesr_sb[:],
                             rhs=sh2[:], start=True, stop=True)
            a2b = stp.tile([P, P], f32)
            nc.vector.tensor_copy(out=a2b[:], in_=bps[:, :P])
            s2b = stp.tile([P, P], f32)
            nc.vector.tensor_copy(out=s2b[:], in_=bps[:, P:])

            # slot maxes -> [slot, feat] rows; apply BN2 affine there
            tp2 = ps_tr.tile([P, 2 * P], f32, tag="sps", space="PSUM")
            nc.tensor.transpose(out=tp2[:, :P], in_=segcols[:],
                                identity=ident[:])
            slotrows = wp.tile([P, P], f32, tag="slotrows")
            nc.vector.tensor_copy(out=slotrows[:], in_=tp2[:, :P])
            nc.vector.tensor_tensor(out=slotrows[:], in0=slotrows[:],
                                    in1=a2b[:], op=mybir.AluOpType.mult)
            nc.vector.tensor_tensor(out=slotrows[:], in0=slotrows[:],
                                    in1=s2b[:], op=mybir.AluOpType.add)
            nc.gpsimd.indirect_dma_start(
                out=segtab[:], out_offset=bass.IndirectOffsetOnAxis(
                    ap=offe_sb[:, :1], axis=0),
                in_=slotrows[:], in_offset=None)
            nc.gpsimd.collective_compute(
                "AllReduce", mybir.AluOpType.max,
                replica_groups=[list(range(NC))],
                ins=[segtab[0:NG, :]], outs=[arout.opt()])
            nc.sync.dma_start(out=out[:], in_=arout[:])

    nc.compile()
    return nc


def _in_maps(x, W1, b1, gamma1, beta1, W2, b2, gamma2, beta2, meta):
    x = np.ascontiguousarray(np.asarray(x, dtype=np.float32))
    iota_b = np.tile(np.arange(P, dtype=np.float32), (P, 1)).astype(
        ml_dtypes.bfloat16)
    onesel = np.ones((P, 2), dtype=np.float32)
    onesel[LASTV:, 1] = 0.0
    assert np.all(np.asarray(gamma2) > 0), "segmax-before-BN2 needs gamma2>0"
    w2 = np.asarray(W2, dtype=np.float32)
    w2p = np.zeros((P, DHID), dtype=np.float32)
    for j in range(2):
        w2p[:, j * P:(j + 1) * P] = w2[j * P:(j + 1) * P, :]
    b2b = np.tile(np.asarray(b2, dtype=np.float32)[None, :], (P, 1))
    common = {
        "x_tab": x,
        "w1": np.asarray(W1, dtype=np.float32),
        "w2p": w2p,
        "b1h": np.asarray(b1, dtype=np.float32).reshape(2, P).T.copy(),
        "b2b": b2b,
        "g1h": np.asarray(gamma1, dtype=np.float32).reshape(2, P).T.copy(),
        "bt1h": np.asarray(beta1, dtype=np.float32).reshape(2, P).T.copy(),
        "g2r": np.asarray(gamma2, dtype=np.float32)[None, :],
        "bt2r": np.asarray(beta2, dtype=np.float32)[None, :],
        "iota_b": iota_b,
        "onesel": onesel,
        "onesrow": np.ones((1, P), dtype=np.float32),
        "onesb": np.ones((1, P), dtype=ml_dtypes.bfloat16),
    }
    maps = []
    for c in range(NC):
        m = dict(common)
        m["x_own"] = x[c * NLOC:(c + 1) * NLOC].astype(ml_dtypes.bfloat16)
        m["src16"] = meta["src_host"][c]
        m["dstlB"] = meta["dstl_host"][c].astype(ml_dtypes.bfloat16)
        m["normA"] = meta["norm_host"][c]
        m["src16b"] = meta["src2_host"][c]
        m["dstlB2"] = meta["dstl2_host"][c].astype(ml_dtypes.bfloat16)
        m["normB2"] = meta["norm2_host"][c].astype(ml_dtypes.bfloat16)
        m["dinv2b"] = meta["dinv2"][c].astype(ml_dtypes.bfloat16)
        m["selidx"] = meta["selidx"][c]
        m["offe"] = meta["offe"][c]
        maps.append(m)
    return maps


def run(inputs, debug=False, trace=False):  # debug kept for test.py compat
    meta = _prep(inputs["x"], inputs["edge_index"], inputs["batch"])
    key = ("v14b", meta["totch"], meta["totch2"], meta["SMAX"],
           tuple(meta["cc"].reshape(-1)), tuple(meta["cc2"].reshape(-1)),
           tuple(meta["scount"]))
    if key not in _cache:
        _cache[key] = _build(meta["cc"], meta["totch"], meta["cc2"],
                             meta["totch2"], meta["SMAX"], meta["scount"])
    nc = _cache[key]
    maps = _in_maps(inputs["x"], inputs["W1"], inputs["b1"],
                    inputs["gamma1"], inputs["beta1"], inputs["W2"],
                    inputs["b2"], inputs["gamma2"], inputs["beta2"], meta)
    res = bass_utils.run_bass_kernel_spmd(
        nc, maps, core_ids=list(range(NC)), trace=trace)
    out = np.array(res.results[0]["out"])
    empty = meta["gcounts"] == 0
    if empty.any():
        out[empty] = -np.inf
    return out, res, meta


def kernel(**inputs) -> np.ndarray:
    out, _, _ = run(inputs)
    return out
